# revision 42
# baseline (speedup 1.0000x reference)
"""Trainium2 Bass kernel for MixedPerformerAttention (B=2,S=2048,D=2048,H=16).

Sharding: 8 cores = 2 batches x 4 head-slots. Core c (b=c//4, j=c%4) owns
performer heads {2j, 2j+1} (kv head j) and softmax heads {8+2j, 8+2j+1}
(kv head 4+j), plus the matching Wq/Wk/Wv rows and Wo columns. Each core
computes a [S, D] partial output projection; the host sums 4 partials/batch.

All matmul operands are bf16 (fp32 PSUM accumulation). The two performer
heads share their GQA kv head, so k-features, pk and the entire kv prefix
state are computed once and reused by both heads; the q-side processes both
heads per chunk through merged [128,256] tiles (h-major) to amortize DVE
per-instruction overhead. Reciprocals run on DVE (vector.reciprocal), and
the performer per-token 1/den broadcast is a rank-1 f32r matmul instead of
gpsimd partition_broadcast.

Schedule (software pipeline): window J emits
  A: Q/K/V projections (dense PE, DMA-paced only in window 0);
  B: softmax scores/AV/den pipeline (ACT-bound: two [128,512] exps/block).
     This window's performer feat+bias units are spread between the first
     blocks (their exps ride the ACT queue just behind the score exps), and
     the PE slack is filled by interleaving the O-projection of window J-1
     plus this window's performer attn/fin units between blocks;
  post-B: softmax_norm (a single joint Ln/Exp over both den rows on psum
     partitions 0/32). For the last window the O-projection runs right
     here, its first block's performer-half matmuls pre-issued start-only
     so the PE chews on them while the Ln/Exp/bb chain resolves.
All big DRAM tensors are staged by the host in the exact SBUF layout so
every DMA is a fully contiguous [128, N] transfer.

The performer branch reproduces the reference's exact stabilizers (per-token
q-stab + per-(b,h) global k-stab) so the EPS=1e-6 denominator guard matches;
stabk is computed on the host at runtime and shipped in `nbinit`.

PSUM (8 banks): pp ring x3 (projections / scores / last-window pre-issued
O-proj), av ring x2 (softmax accumulators + performer feature tiles),
sm ring x2 (transposes/aT/kvc/num/bb/pso), dacc x1 (softmax den rows on
partitions 0 and 32 of one bank).
"""

import sys

sys.path.insert(0, "/opt/trn_rl_repo")

import numpy as np

import concourse.bass as bass
import concourse.tile as tile
from concourse import bacc, mybir
from concourse._compat import with_exitstack

F32 = mybir.dt.float32
F32R = mybir.dt.float32r
BF16 = mybir.dt.bfloat16
AF = mybir.ActivationFunctionType
AX = mybir.AxisListType
ALU = mybir.AluOpType

B, S, D = 2, 2048, 2048
H, KVH, HD = 16, 8, 128
NPH, M, C = 8, 128, 128
SCALE = HD ** -0.5
EPS = 1e-6
LNM = float(np.log(np.sqrt(M)))
HDQ = HD ** -0.25

NJ, JW, ND, NB = 4, 512, 16, 16


@with_exitstack
def _emit(ctx, tc, aps, debug=False):
    nc = tc.nc
    hsT, wq, wk, wv, wo = aps["hsT"], aps["wq"], aps["wk"], aps["wv"], aps["wo"]
    out = aps["out"]

    pers = ctx.enter_context(tc.tile_pool(name="pers", bufs=1))
    hst_p = ctx.enter_context(tc.tile_pool(name="hst", bufs=2))
    rot_p = ctx.enter_context(tc.tile_pool(name="rot", bufs=2))
    qt_p = ctx.enter_context(tc.tile_pool(name="qt", bufs=2))
    pt_p = ctx.enter_context(tc.tile_pool(name="pt", bufs=2))
    at_p = ctx.enter_context(tc.tile_pool(name="at", bufs=2))
    sm_p = ctx.enter_context(tc.tile_pool(name="sm", bufs=2))
    ost_p = ctx.enter_context(tc.tile_pool(name="ost", bufs=2))
    psp = ctx.enter_context(tc.tile_pool(name="psp", bufs=1, space="PSUM"))

    def ppt(shape=None, tag="pp"):
        return psp.tile(shape or [128, JW], F32, name=tag, tag="pp", bufs=3)

    def smt(shape, dt, name):
        return psp.tile(shape, dt, name=name, tag="sm", bufs=2)

    mm = nc.tensor.matmul

    # ---- weights + J0 activations. All big tensors are staged in DRAM by
    # the host in the exact SBUF layout, so every DMA below is a fully
    # contiguous [128, N] transfer. J0 computes V->K->Q, so the stream order
    # is: wv (split in two so the first vproj d-steps start at ~1MB landed),
    # then hst, then wk/wq arriving while V/K compute ----
    wq_t = pers.tile([128, ND * 512], BF16, name="wq_t", tag="wq")
    wk_t = pers.tile([128, ND * 256], BF16, name="wk_t", tag="wk")
    wv_t = pers.tile([128, ND * 256], BF16, name="wv_t", tag="wv")
    wo_t = pers.tile([128, 4 * D], BF16, name="wo_t", tag="wo")
    hst0 = hst_p.tile([128, ND * JW], BF16, name="hst", tag="hst")
    nc.sync.dma_start(wv_t[:, 0:512], wv[:, 0:512])
    nc.sync.dma_start(hst0[:, 0:1024], hsT[:, 0:1024])
    nc.sync.dma_start(wv_t[:, 512:2048], wv[:, 512:2048])
    nc.sync.dma_start(hst0[:, 1024:2048], hsT[:, 1024:2048])
    nc.sync.dma_start(wv_t[:, 2048:4096], wv[:, 2048:4096])
    for dd in range(1, 4):
        nc.sync.dma_start(hst0[:, dd * 2048:(dd + 1) * 2048],
                          hsT[:, dd * 2048:(dd + 1) * 2048])
    nc.sync.dma_start(wk_t[:], wk[:])
    co0 = rot_p.tile([128, JW], BF16, name="cos", tag="cos")
    si0 = rot_p.tile([128, JW], BF16, name="sin", tag="sin")
    nc.sync.dma_start(co0[:], aps["cost"][:, 0:JW])
    nc.sync.dma_start(si0[:], aps["sintn"][:, 0:JW])
    nc.sync.dma_start(wq_t[:], wq[:])
    omgx = pers.tile([128, 128], BF16, name="omgx", tag="omgx")
    nc.sync.dma_start(omgx[:], aps["omgx"][:])
    cons2 = pers.tile([128, 2], BF16, name="cons2", tag="cons2")
    nc.sync.dma_start(cons2[:], aps["cons2"][:])
    ident = pers.tile([128, 128], BF16, name="ident", tag="ident")
    nc.sync.dma_start(ident[:], aps["ident"][:])
    trimask = pers.tile([128, 128], BF16, name="trimask", tag="trimask")
    nc.sync.dma_start(trimask[:], aps["trimask"][:])
    onescol = pers.tile([128, 1], BF16, name="onescol", tag="onescol")
    nc.sync.dma_start(onescol[:], aps["onescol"][:])
    onesr = pers.tile([33, 128], BF16, name="onesr", tag="onesr")
    nc.sync.dma_start(onesr[:], aps["onesr"][:])
    nbinit = pers.tile([128, 4], F32, name="nbinit", tag="nbinit")
    nc.sync.dma_start(nbinit[:], aps["nbinit"][:])
    nc.sync.dma_start(wo_t[:], wo[:])

    # ---- persistent K/V and performer state ----
    ktp = pers.tile([128, 2048], BF16, name="ktp", tag="ktp")
    kts = pers.tile([128, 2048], BF16, name="kts", tag="kts")
    vp = [pers.tile([128, 132], BF16, name=f"vp{i}", tag=f"vp{i}") for i in range(NB)]
    vs = [pers.tile([128, 128], BF16, name=f"vs{i}", tag=f"vs{i}") for i in range(NB)]
    for i in range(NB):
        nc.vector.memset(vp[i][:, 128:129], 1.0)
    kv_bf = sm_p.tile([128, 132], BF16, name="kvbf", tag="kvbf", bufs=2)
    nc.vector.memset(kv_bf[:, 0:129], 0.0)

    # softmax denominator rows, partitions 0/32 (base must be 0/32/64),
    # sharing a single psum bank
    dacc = psp.tile([33, 512], F32, name="dacc", tag="dacc", bufs=1)
    dn_sl = [dacc[0:1, :], dacc[32:33, :]]

    def rotary(ps, dst, co, si):
        # dst = ps*cos + rot_half(ps)*sin; sintn has [-s; s] baked in. The
        # half-swapped products read PSUM directly (mixed PSUM/SB operands
        # are exempt from the same-base-partition rule).
        tmp = rot_p.tile([128, JW], BF16, name="rtmp", tag="rtmp", bufs=2)
        nc.vector.tensor_mul(tmp[0:64, :], ps[64:128, :], si[0:64, :])
        nc.vector.tensor_mul(tmp[64:128, :], ps[0:64, :], si[64:128, :])
        pc = rot_p.tile([128, JW], BF16, name="pc", tag="pc", bufs=2)
        nc.scalar.copy(pc[:], ps[:])
        nc.vector.tensor_mul(dst, pc[:], co[:])
        nc.vector.tensor_add(dst, dst, tmp[:])

    def oproj_sb(Jp, at3, sb, act_copies=False):
        # copies default to DVE (oproj mostly runs inside B sections where
        # ACT is saturated by the softmax exps); the tail splits them with
        # ACT, which idles there while DVE carries the performer chains
        atp01p, ats0p, ats1p = at3
        s0p = Jp * JW
        o = ost_p.tile([128, D], BF16, name="ost", tag="ost", bufs=2)
        asl = [atp01p[:, sb * 128:sb * 128 + 128],
               atp01p[:, 512 + sb * 128:512 + sb * 128 + 128],
               ats0p[:, sb * 128:(sb + 1) * 128],
               ats1p[:, sb * 128:(sb + 1) * 128]]
        for oc in range(4):
            pso = smt([128, JW], F32, "pso")
            for i in range(4):
                mm(pso[:], asl[i], wo_t[:, i * D + oc * 512:i * D + (oc + 1) * 512],
                   start=(i == 0), stop=(i == 3))
            if act_copies and oc % 2 == 1:
                nc.scalar.copy(o[:, oc * 512:(oc + 1) * 512], pso[:])
            else:
                nc.vector.tensor_copy(o[:, oc * 512:(oc + 1) * 512], pso[:])
            if oc % 2 == 1:
                half = slice((oc - 1) * 512, (oc + 1) * 512)
                nc.sync.dma_start(
                    out[s0p + sb * 128:s0p + (sb + 1) * 128, half], o[:, half])

    # ---- performer units for window J, run during window J+1's A section
    # (or inline in the tail for the last window) ----
    def make_cunits(J, qt01, atp01):
        s0 = J * JW
        feat = {}
        bias_d = {}
        att_d = {}
        q2J = qt_p.tile([128, 1024], BF16, name="q2J", tag="q2J", bufs=2)
        k2J = qt_p.tile([128, JW], BF16, name="k2J", tag="k2J", bufs=2)

        def c_feat(t):
            # feat tiles ride the "av" psum ring (idle during A sections) so
            # the pp ring never couples projections to the bias ACT chain
            c = 4 * J + t
            fq01 = psp.tile([128, 264], F32, name="fq01", tag="av", bufs=2)
            for h in range(2):
                qo = h * 512 + t * 128
                mm(fq01[:, h * 132:h * 132 + 128], qt01[:, qo:qo + 128],
                   omgx[:], start=True, stop=True)
                mm(fq01[:, h * 132 + 128:h * 132 + 130], q2J[:, qo:qo + 128],
                   cons2[:], start=True, stop=True)
            fk = psp.tile([128, 132], F32, name="fk", tag="av", bufs=2)
            mm(fk[:, 0:128], ktp[:, c * 128:(c + 1) * 128], omgx[:],
               start=True, stop=True)
            mm(fk[:, 128:130], k2J[:, t * 128:(t + 1) * 128], cons2[:],
               start=True, stop=True)
            feat[t] = (fq01, fk)

        def c_bias(t):
            fq01, fk = feat.pop(t)
            f3 = fq01.rearrange("p (h c) -> p h c", h=2)
            nmax = sm_p.tile([128, 2], F32, name="nmax", tag="nmax", bufs=2)
            nc.vector.tensor_reduce(nmax[:], f3[:, :, 0:128], axis=AX.X,
                                    op=ALU.max, negate=True)
            nbq = sm_p.tile([128, 2], F32, name="nbq", tag="nbq", bufs=2)
            nc.vector.tensor_tensor(nbq[:], nmax[:],
                                    f3[:, :, 128:129].squeeze(-1),
                                    op=ALU.subtract)
            nc.vector.tensor_scalar(nbq[:], nbq[:], 1.0, -LNM,
                                    ALU.mult, ALU.add)
            nbk = sm_p.tile([128, 1], F32, name="nbk", tag="nbk", bufs=2)
            nc.vector.tensor_scalar(nbk[:], fk[:, 128:129], -1.0,
                                    nbinit[:, 0:1], ALU.mult, ALU.add)
            pq01 = sm_p.tile([128, 256], BF16, name="pq01", tag="pq01", bufs=4)
            for h in range(2):
                nc.scalar.activation(pq01[:, h * 128:(h + 1) * 128],
                                     fq01[:, h * 132:h * 132 + 128], AF.Exp,
                                     bias=nbq[:, h:h + 1], scale=1.0)
            pk = sm_p.tile([128, 128], BF16, name="pk", tag="pk", bufs=4)
            nc.scalar.activation(pk[:], fk[:, 0:128], AF.Exp, bias=nbk[:],
                                 scale=1.0)
            bias_d[t] = (pq01, pk)

        def c_attn(t):
            # token-major numerators/denominators: the per-token divide is a
            # [128,2] column reciprocal + per-partition tensor_scalar, then
            # two PE transposes bring the result back to feature-major.
            nonlocal kv_bf
            c = 4 * J + t
            pq01, pk = bias_d.pop(t)
            trq01 = smt([128, 256], BF16, "trq")
            for h in range(2):
                nc.tensor.transpose(trq01[:, h * 128:(h + 1) * 128],
                                    pq01[:, h * 128:(h + 1) * 128], ident[:])
            trk = smt([128, 128], BF16, "trk")
            nc.tensor.transpose(trk[:], pk[:], ident[:])
            pqT01 = sm_p.tile([128, 256], BF16, name="pqT01", tag="pqT01", bufs=2)
            nc.vector.tensor_copy(pqT01[:], trq01[:])
            pkT = sm_p.tile([128, 128], BF16, name="pkT", tag="pkT", bufs=2)
            nc.vector.tensor_copy(pkT[:], trk[:])
            kvc = smt([128, 132], F32, "kvc")
            mm(kvc[:, 0:129], pk[:], vp[c][:, 0:129], start=True, stop=True)
            aT01 = smt([128, 256], F32, "aT")
            mm(aT01[:], pkT[:], pqT01[:], start=True, stop=True)
            aM01 = sm_p.tile([128, 256], BF16, name="aM01", tag="aM01", bufs=2)
            nc.vector.tensor_tensor(
                aM01.rearrange("p (h q) -> p h q", h=2),
                aT01.rearrange("p (h q) -> p h q", h=2),
                trimask.unsqueeze(1).broadcast_to([128, 2, 128]),
                op=ALU.mult)
            numt = smt([128, 256], F32, "numt")      # [q, h*hd] token-major
            dnpt = smt([128, 2], F32, "dnpt")        # [q, h] token-major
            for h in range(2):
                hs_ = slice(h * 128, (h + 1) * 128)
                mm(numt[:, hs_], aM01[:, hs_], vp[c][:, 0:128],
                   start=True, stop=False)
                mm(dnpt[:, h:h + 1], aM01[:, hs_], onescol[:],
                   start=True, stop=False)
                mm(numt[:, hs_], pqT01[:, hs_], kv_bf[:, 0:128],
                   start=False, stop=True)
                mm(dnpt[:, h:h + 1], pqT01[:, hs_], kv_bf[:, 128:129],
                   start=False, stop=True)
            nkv = sm_p.tile([128, 132], BF16, name="kvbf", tag="kvbf", bufs=2)
            nc.vector.tensor_add(nkv[:, 0:129], kv_bf[:, 0:129], kvc[:, 0:129])
            kv_bf = nkv
            numc = sm_p.tile([128, 256], BF16, name="numc", tag="numc", bufs=2)
            nc.vector.tensor_copy(numc[:], numt[:])
            dent = sm_p.tile([128, 2], F32, name="dent", tag="dent", bufs=2)
            nc.vector.tensor_scalar(dent[:], dnpt[:], 1.0, nbinit[:, 2:3],
                                    ALU.mult, ALU.add)
            nc.vector.reciprocal(dent[:], dent[:])
            att = sm_p.tile([128, 256], BF16, name="att", tag="att", bufs=2)
            for h in range(2):
                hs_ = slice(h * 128, (h + 1) * 128)
                nc.vector.tensor_scalar_mul(att[:, hs_], numc[:, hs_],
                                            dent[:, h:h + 1])
            att_d[t] = att

        def c_fin(t):
            # transpose token-major attention back to feature-major atp01
            att = att_d.pop(t)
            cs = t * 128
            atr = smt([128, 256], BF16, "atr")
            for h in range(2):
                nc.tensor.transpose(atr[:, h * 128:(h + 1) * 128],
                                    att[:, h * 128:(h + 1) * 128], ident[:])
            nc.vector.tensor_copy(
                atp01.rearrange("p (h s) -> p h s", h=2)[:, :, cs:cs + 128],
                atr.rearrange("p (h q) -> p h q", h=2))

        def u1(t):
            if t == 0:
                nc.vector.tensor_mul(q2J[:], qt01[:], qt01[:])
                nc.vector.tensor_mul(k2J[:], ktp[:, s0:s0 + JW],
                                     ktp[:, s0:s0 + JW])
            c_feat(t)
            c_bias(t)

        def u2(t):
            c_attn(t)

        def u3(t):
            c_fin(t)

        return u1, [(u2, 0), (u3, 0), (u2, 1), (u3, 1), (u2, 2), (u3, 2),
                    (u2, 3), (u3, 3)]

    prev_at = None   # (atp01, ats0, ats1) of window J-1, O-projected in B

    for J in range(NJ):
        s0 = J * JW
        if J == 0:
            hst, co, si = hst0, co0, si0
        else:
            hst, co, si = hst_n, co_n, si_n

        # ================= A: projections (+ performer units of J-1) ======
        qt01 = qt_p.tile([128, 1024], BF16, name="qt01", tag="qt01", bufs=2)
        qt2 = qt_p.tile([128, JW], BF16, name="qt2", tag="qt2", bufs=2)
        qt3 = qt_p.tile([128, JW], BF16, name="qt3", tag="qt3", bufs=2)
        qdst = [qt01[:, 0:512], qt01[:, 512:1024], qt2[:], qt3[:]]

        # B-section state, defined before A so J0 can pre-emit its first two
        # score blocks between projection groups (J0 has no other B filler)
        nblk = 4 * J + 4
        av = [psp.tile([128, JW], F32, name=f"av{h}", tag="av", bufs=2)
              for h in range(2)]
        pts = {}

        def st_exp(i):
            t = i - 4 * J  # >= 0 on diagonal blocks
            q0 = max(t, 0) * 128
            for h in range(2):
                st = ppt()
                mm(st[:, q0:JW], kts[:, i * 128:(i + 1) * 128],
                   (qt2 if h == 0 else qt3)[:, q0:JW], start=True, stop=True)
                pth = pt_p.tile([128, JW], BF16, name=f"pt{h}", tag=f"pt{h}",
                                bufs=2)
                nc.scalar.activation(pth[:, q0:JW], st[:, q0:JW], AF.Exp,
                                     bias=0.0, scale=SCALE)
                if t >= 0:
                    nc.vector.tensor_mul(pth[:, q0:q0 + 128],
                                         pth[:, q0:q0 + 128], trimask[:])
                pts[(i, h)] = (pth, q0)

        def av_dn(i):
            for h in range(2):
                pth, q0 = pts.pop((i, h))
                mm(av[h][:, q0:JW], vs[i][:], pth[:, q0:JW],
                   start=(i == 0), stop=(i == nblk - 1))
                mm(dn_sl[h][:, q0:JW], onescol[:], pth[:, q0:JW],
                   start=(i == 0), stop=(i == nblk - 1))

        def qproj_g(g, hst=hst, co=co, si=si, qdst=qdst):
            ps = ppt()
            for d in range(ND):
                mm(ps[:], wq_t[:, d * 512 + g * 128:d * 512 + (g + 1) * 128],
                   hst[:, d * JW:(d + 1) * JW],
                   start=(d == 0), stop=(d == ND - 1))
            rotary(ps, qdst[g], co, si)

        def kproj_g(g, hst=hst, co=co, si=si, s0=s0):
            ps = ppt()
            for d in range(ND):
                mm(ps[:], wk_t[:, d * 256 + g * 128:d * 256 + (g + 1) * 128],
                   hst[:, d * JW:(d + 1) * JW],
                   start=(d == 0), stop=(d == ND - 1))
            kt = ktp if g == 0 else kts
            rotary(ps, kt[:, s0:s0 + JW], co, si)

        def vproj_b(sb, J=J, hst=hst):
            blk = 4 * J + sb
            ps = ppt([128, 256])
            for d in range(ND):
                mm(ps[:], hst[:, d * JW + sb * 128:d * JW + (sb + 1) * 128],
                   wv_t[:, d * 256:(d + 1) * 256], start=(d == 0),
                   stop=(d == ND - 1))
            nc.scalar.copy(vp[blk][:, 0:128], ps[:, 0:128])
            nc.scalar.copy(vs[blk][:], ps[:, 128:256])

        if J > 0:
            for g in range(4):
                qproj_g(g)
            for g in range(2):
                kproj_g(g)
            for sb in range(4):
                vproj_b(sb)
        else:
            # window 0 is DMA-paced: V (fed by the hst stream itself) and K
            # run while the Q weights are still arriving
            for sb in range(4):
                vproj_b(sb)
            for g in range(2):
                kproj_g(g)
            for g in range(4):
                qproj_g(g)

        # prefetch next window's activations while B runs
        if J + 1 < NJ:
            s1 = (J + 1) * JW
            hst_n = hst_p.tile([128, ND * JW], BF16, name="hst", tag="hst")
            nc.sync.dma_start(hst_n[:],
                              hsT[:, (J + 1) * 8192:(J + 2) * 8192])
            co_n = rot_p.tile([128, JW], BF16, name="cos", tag="cos")
            si_n = rot_p.tile([128, JW], BF16, name="sin", tag="sin")
            nc.sync.dma_start(co_n[:], aps["cost"][:, s1:s1 + JW])
            nc.sync.dma_start(si_n[:], aps["sintn"][:, s1:s1 + JW])

        # ================= B: softmax heads =================
        # This window's performer feat+bias units are spread between the
        # first B blocks: each u1's exps land just behind the score exps in
        # the ACT queue, early enough that the u2/u3 fillers further down
        # never wait on a feat->bias chain, but without damming B's pipe.
        atp01 = at_p.tile([128, 1024], BF16, name="atp01", tag="atp01", bufs=2)
        u1_J, rest_J = make_cunits(J, qt01, atp01)

        # B's blocks are ACT-bound (two [128,512] exps/block vs ~1.28us of
        # PE), so the O-projection of window J-1 and this window's performer
        # u2/u3 units (whose exps already ran above) interleave between
        # blocks as PE filler, spread evenly across the nblk slots.
        seq = []
        for t in range(4):
            if prev_at is not None:
                seq.append(lambda t=t: oproj_sb(J - 1, prev_at, t))
            seq.append(lambda t=t: rest_J[2 * t][0](t))      # u2(t)
            seq.append(lambda t=t: rest_J[2 * t + 1][0](t))  # u3(t)
        L = len(seq)
        nfill = min(nblk, L)  # fillers finish early when slots outnumber them
        u1_J(0)
        st_exp(0)
        u1_J(1)
        for i in range(1, nblk):
            st_exp(i)
            if i + 1 <= 3:
                u1_J(i + 1)
            av_dn(i - 1)
            for f in seq[((i - 1) * L) // nfill:(i * L) // nfill]:
                f()
        av_dn(nblk - 1)
        for f in seq[((nblk - 1) * L) // nfill:]:
            f()
        avcs = []
        for h in range(2):
            avc = sm_p.tile([128, JW], BF16, name="avc", tag="avc", bufs=2)
            nc.vector.tensor_copy(avc[:], av[h][:])
            avcs.append(avc)

        # ==== post-B: softmax_norm(J). Both heads' Ln/Exp run as single ACT
        # calls over partitions 0..32 (the den rows), halving the chain. ====
        def softmax_norm():
            r = sm_p.tile([33, JW], F32, name="rcs", tag="rcs", bufs=2)
            nc.scalar.activation(r[:], dacc[:], AF.Ln, bias=0.0, scale=1.0)
            rb = sm_p.tile([33, JW], BF16, name="rcb", tag="rcb", bufs=2)
            nc.scalar.activation(rb[:], r[:], AF.Exp, bias=0.0, scale=-1.0)
            res = []
            for h in range(2):
                bb = smt([128, JW], F32, "bbs")
                mm(bb[:], onesr[32 * h:32 * h + 1, :],
                   rb[32 * h:32 * h + 1, :], start=True, stop=True)
                a = at_p.tile([128, JW], BF16, name=f"ats{h}", tag=f"ats{h}",
                              bufs=2)
                nc.vector.tensor_mul(a[:], avcs[h][:], bb[:])
                res.append(a)
            return res

        if J < NJ - 1:
            ats = softmax_norm()
            prev_at = (atp01, ats[0], ats[1])
        else:
            # tail: only this window's O-projection remains. sb0's performer
            # (atp01) accumulation halves are issued start-only — two psos on
            # the now-idle pp ring, two on the av ring — so the PE chews on
            # them while softmax_norm's Ln/Exp/bb chain resolves; the ats
            # halves then land with stop flags, and sb1-3 follow normally.
            psos = []
            for oc in range(4):
                if oc < 2:
                    pso = ppt()
                else:
                    pso = psp.tile([128, JW], F32, name="psoav", tag="av",
                                   bufs=2)
                for i in range(2):
                    mm(pso[:], atp01[:, i * 512:i * 512 + 128],
                       wo_t[:, i * D + oc * 512:i * D + (oc + 1) * 512],
                       start=(i == 0), stop=False)
                psos.append(pso)
            ats = softmax_norm()
            at3 = (atp01, ats[0], ats[1])
            s0p = J * JW
            o = ost_p.tile([128, D], BF16, name="ost", tag="ost", bufs=2)
            for oc in range(4):
                pso = psos[oc]
                for k in range(2):
                    mm(pso[:], ats[k][:, 0:128],
                       wo_t[:, (2 + k) * D + oc * 512:(2 + k) * D + (oc + 1) * 512],
                       start=False, stop=(k == 1))
                if oc % 2 == 0:
                    nc.vector.tensor_copy(o[:, oc * 512:(oc + 1) * 512], pso[:])
                else:
                    nc.scalar.copy(o[:, oc * 512:(oc + 1) * 512], pso[:])
                    half = slice((oc - 1) * 512, (oc + 1) * 512)
                    nc.sync.dma_start(out[s0p:s0p + 128, half], o[:, half])
            for sb in range(1, 4):
                oproj_sb(J, at3, sb, act_copies=True)


def _pin_act_tables():
    """Make every ACT table-set except natural_log_exp_and_others ineligible so
    the loader never thrashes between the exp-only and ln-only sets."""
    import concourse.bacc as bacc_mod
    if getattr(bacc_mod, "_act_tables_pinned", False):
        return
    orig = bacc_mod.get_activation_tables

    def patched(arch):
        t = orig(arch)
        return {k: (v if k == "natural_log_exp_and_others" else set())
                for k, v in t.items()}

    bacc_mod.get_activation_tables = patched
    bacc_mod._act_tables_pinned = True


def build(debug=False):
    _pin_act_tables()
    nc = bacc.Bacc("TRN2", target_bir_lowering=False, debug=False, num_devices=8)
    shapes = {
        "hsT": [128, NJ * ND * JW], "wq": [128, ND * 512],
        "wk": [128, ND * 256], "wv": [128, ND * 256],
        "wo": [128, 4 * D], "cost": [128, S], "sintn": [128, S],
        "omgx": [128, 128], "cons2": [128, 2], "ident": [128, 128],
        "trimask": [128, 128], "onescol": [128, 1],
    }
    aps = {n: nc.dram_tensor(n, s, BF16, kind="ExternalInput").ap()
           for n, s in shapes.items()}
    aps["nbinit"] = nc.dram_tensor("nbinit", [128, 4], F32,
                                   kind="ExternalInput").ap()
    aps["onesr"] = nc.dram_tensor("onesr", [33, 128], BF16,
                                  kind="ExternalInput").ap()
    aps["out"] = nc.dram_tensor("out", [S, D], BF16, kind="ExternalOutput").ap()
    with tile.TileContext(nc) as tc:
        _emit(tc, aps, debug=debug)
    nc.compile()
    return nc


def host_prep(hidden_states, cos, sin, Wq, Wk, Wv, Wo, omega):
    """Slice/transpose/cast full inputs into 8 per-core input maps."""
    import ml_dtypes
    bf = ml_dtypes.bfloat16
    f32 = np.float32
    hs = np.asarray(hidden_states, f32)
    cos = np.asarray(cos, f32)
    sin = np.asarray(sin, f32)
    Wq, Wk, Wv, Wo = (np.asarray(x, f32) for x in (Wq, Wk, Wv, Wo))
    omega = np.asarray(omega, f32)

    omgx = np.ascontiguousarray((omega * HDQ).T).astype(bf)       # [hd, m]
    cons2 = np.zeros((128, 2), f32)
    cons2[:, 0] = 0.5 * HD ** -0.5
    cons2 = cons2.astype(bf)
    ident = np.eye(128, dtype=f32).astype(bf)
    pidx = np.arange(128)[:, None]
    qidx = np.arange(128)[None, :]
    trimask = (qidx >= pidx).astype(f32).astype(bf)                # keep q>=k
    onescol = np.ones((128, 1), f32).astype(bf)

    # stabk per (b, perf kv head j): max over (s,m) of projk (pre-stab)
    stab = np.zeros((B, 4), f32)
    kproj = np.einsum("bsd,od->bso", hs, Wk[0:512]).reshape(B, S, 4, HD)
    khalf = np.concatenate([-kproj[..., 64:], kproj[..., :64]], axis=-1)
    krot = kproj * cos[:, :, None, :] + khalf * sin[:, :, None, :]
    for b in range(B):
        for j in range(4):
            pj = (krot[b, :, j] * HDQ) @ omega.T
            stab[b, j] = pj.max()

    # SBUF-layout staging: hsl[b][p, J*8192 + d*512 + s] = hs[b][J*512+s, d*128+p]
    hsl = [np.ascontiguousarray(
               hs[b].reshape(NJ, JW, 16, 128).transpose(3, 0, 2, 1)
           ).reshape(128, -1).astype(bf) for b in range(B)]
    costl = [np.ascontiguousarray(cos[b].T).astype(bf) for b in range(B)]
    sintnl = []
    for b in range(B):
        sh = sin[b, :, 0:64]
        sintnl.append(np.ascontiguousarray(
            np.concatenate([-sh, sh], axis=1).T).astype(bf))

    def wlay(rows_T):  # rows_T: [cols, D] -> [128, 16*cols]
        c = rows_T.shape[0]
        return np.ascontiguousarray(
            rows_T.reshape(c, 16, 128).transpose(2, 1, 0)).reshape(128, -1) \
            .astype(bf)

    in_maps = []
    for core in range(8):
        b, j = divmod(core, 4)
        heads = [2 * j, 2 * j + 1, 8 + 2 * j, 8 + 2 * j + 1]
        qrows = np.concatenate([Wq[h * 128:(h + 1) * 128] for h in heads])
        kvh = [j, 4 + j]
        krows = np.concatenate([Wk[g * 128:(g + 1) * 128] for g in kvh])
        vrows = np.concatenate([Wv[g * 128:(g + 1) * 128] for g in kvh])
        wocols = np.concatenate([Wo[:, h * 128:(h + 1) * 128] for h in heads],
                                axis=1)
        # wol[p, i*2048 + c] = wocols[c, i*128+p]
        wol = np.ascontiguousarray(
            wocols.reshape(D, 4, 128).transpose(2, 1, 0)).reshape(128, -1) \
            .astype(bf)
        nbinit = np.zeros((128, 4), f32)
        nbinit[:, 0] = -(stab[b, j] + LNM)
        nbinit[:, 2] = EPS
        in_maps.append({
            "hsT": hsl[b],
            "wq": wlay(qrows),
            "wk": wlay(krows),
            "wv": wlay(vrows),
            "wo": wol,
            "cost": costl[b],
            "sintn": sintnl[b],
            "omgx": omgx, "cons2": cons2, "ident": ident,
            "trimask": trimask, "onescol": onescol,
            "nbinit": nbinit,
            "onesr": np.ones((33, 128), f32).astype(bf),
        })
    return in_maps


_NC_CACHE = {}


def kernel(**inputs):
    from concourse.bass_utils import run_bass_kernel_spmd
    if "nc" not in _NC_CACHE:
        _NC_CACHE["nc"] = build(debug=False)
    nc = _NC_CACHE["nc"]
    in_maps = host_prep(**inputs)
    res = run_bass_kernel_spmd(nc, in_maps, core_ids=list(range(8)))
    out = np.zeros((B, S, D), np.float32)
    for core in range(8):
        out[core // 4] += res.results[core]["out"].astype(np.float32)
    return out
